# revision 7
# baseline (speedup 1.0000x reference)
"""Causal self-attention Trainium2 kernel.

Sharding: 8 cores = 4 batches x 2 head-groups (8 heads each).
Per-core dataflow (everything "transposed" so the contraction dim sits on
SBUF partitions):
  qT/kT  [64*2, T]  = Wq/Wk-slice.T-as-lhsT  @ xT          (fp32r matmuls)
  V      [T, 64*8]  = xT-as-lhsT @ Wv-slice, stored per (head, ktile) as
                      [128, 65] with a ones column (row-sum trick)
  ST     [k, q]     = kT-as-lhsT @ qT   (two heads row-packed in PE array)
  PT     = exp(ST/8) with causal lower-left structure (upper blocks skipped,
           diagonal blocks masked on GPSIMD)
  OT[65, q] += Vaug-as-lhsT @ PT        (row 64 accumulates softmax sums)
  outT   = OT[0:64] * bcast(1/OT[64])
  y      = outT-as-lhsT @ Wo-row-slice  (partial; host sums the 2 head-groups)
"""
import numpy as np

B, T, D, H = 4, 2048, 1024, 16
HD = D // H            # 64
NCORES = 8
HPC = 8                # heads per core
FPC = HPC * HD         # 512 feature cols per core
NPAIR = HPC // 2       # 4 head pairs
ND = D // 128          # 8 contraction tiles
KT = T // 128          # 16 k-tiles
NCH = T // 512         # 4 q-chunks

_CACHE = {}


def _build():
    import concourse.mybir as mybir
    import concourse.tile as tile
    from concourse import bacc

    f32 = mybir.dt.float32
    f32r = mybir.dt.float32r
    Exp = mybir.ActivationFunctionType.Exp
    mult = mybir.AluOpType.mult

    nc = bacc.Bacc("TRN2", target_bir_lowering=False, debug=False,
                   num_devices=NCORES)
    xT = nc.dram_tensor("xT", [D, T], f32, kind="ExternalInput")
    Wq = nc.dram_tensor("Wq", [D, FPC], f32, kind="ExternalInput")
    Wk = nc.dram_tensor("Wk", [D, FPC], f32, kind="ExternalInput")
    Wv = nc.dram_tensor("Wv", [D, FPC], f32, kind="ExternalInput")
    bq = nc.dram_tensor("bq", [FPC], f32, kind="ExternalInput")
    bk = nc.dram_tensor("bk", [FPC], f32, kind="ExternalInput")
    bv = nc.dram_tensor("bv", [FPC], f32, kind="ExternalInput")
    Wo = nc.dram_tensor("Wo", [FPC, D], f32, kind="ExternalInput")
    y = nc.dram_tensor("y", [T, D], f32, kind="ExternalOutput")

    VSTR = KT * 65     # per-head stride in Vaug free dim

    from contextlib import ExitStack
    with tile.TileContext(nc) as tc, ExitStack() as es:
        pers = es.enter_context(tc.tile_pool(name="pers", bufs=1))
        wts = tc.tile_pool(name="wts", bufs=1)
        wpool = wts.__enter__()
        if True:
            # ---- resident tiles ----
            wq_sb = [wpool.tile([128, FPC], f32r, tag=f"wq{d}", name=f"wq{d}") for d in range(ND)]
            wk_sb = [wpool.tile([128, FPC], f32r, tag=f"wk{d}", name=f"wk{d}") for d in range(ND)]
            wv_sb = [wpool.tile([128, FPC], f32r, tag=f"wv{d}", name=f"wv{d}") for d in range(ND)]
            qT = [pers.tile([128, T], f32r, tag=f"qT{p}", name=f"qT{p}") for p in range(NPAIR)]
            kTt = [pers.tile([128, T], f32r, tag=f"kT{p}", name=f"kT{p}") for p in range(NPAIR)]
            vaug = pers.tile([128, HPC * VSTR], f32r, tag="vaug")
            bq_sb = wpool.tile([128, NPAIR], f32, tag="bq")
            bk_sb = wpool.tile([128, NPAIR], f32, tag="bk")
            bv_row = wpool.tile([1, FPC], f32, tag="bvr")
            bv_bc = wpool.tile([128, FPC], f32, tag="bvb")

            for d in range(ND):
                nc.gpsimd.dma_start(wq_sb[d][:], Wq[128 * d:128 * (d + 1), :])
                nc.gpsimd.dma_start(wk_sb[d][:], Wk[128 * d:128 * (d + 1), :])
                nc.gpsimd.dma_start(wv_sb[d][:], Wv[128 * d:128 * (d + 1), :])
            nc.sync.dma_start(bq_sb[:], bq[:].rearrange("(t p) -> p t", p=128))
            nc.sync.dma_start(bk_sb[:], bk[:].rearrange("(t p) -> p t", p=128))
            nc.sync.dma_start(bv_row[:], bv[:])
            nc.gpsimd.partition_broadcast(bv_bc[:], bv_row[:])
            # ones columns of vaug (col 64 of each [128,65] block).
            # memset can't encode f32r, so memset f32 then cast-copy on DVE.
            ones_f32 = wpool.tile([128, KT], f32, tag="ones1", name="ones_f32")
            nc.vector.memset(ones_f32[:], 1.0)
            for h in range(HPC):
                ones_ap = vaug[:, VSTR * h:VSTR * (h + 1)].rearrange(
                    "p (k x) -> p k x", x=65)[:, :, 64:65]
                nc.vector.tensor_copy(ones_ap, ones_f32[:].rearrange(
                    "p (k x) -> p k x", x=1))

            # ================= phase B: qkv projections =================
            def qkv_half(xpool, half):
                xs = [xpool.tile([128, T // 2], f32r, tag=f"x{half}_{d}", name=f"x{half}_{d}")
                      for d in range(ND)]
                for d in range(ND):
                    nc.gpsimd.dma_start(
                        xs[d][:], xT[128 * d:128 * (d + 1),
                                     half * (T // 2):(half + 1) * (T // 2)])
                with tc.tile_pool(name=f"psB{half}", bufs=3, space="PSUM") as pp:
                    # V: out[t, dv] ; lhsT = xT tile, rhs = Wv tile
                    for tj in range(8 * half, 8 * (half + 1)):
                        ps = pp.tile([128, FPC], f32, tag="ps", name="psv")
                        lo = tj * 128 - half * (T // 2)
                        for d in range(ND):
                            nc.tensor.matmul(ps[:], xs[d][:, lo:lo + 128],
                                             wv_sb[d][:],
                                             start=(d == 0), stop=(d == ND - 1))
                        # scatter per-head into vaug [p, 8, 64] + bias
                        dst = vaug[:].rearrange("p (h z) -> p h z", h=HPC)[
                            :, :, 65 * tj:65 * tj + HD]
                        src = ps[:].rearrange("p (h x) -> p h x", h=HPC)
                        bsrc = bv_bc[:].rearrange("p (h x) -> p h x", h=HPC)
                        nc.vector.tensor_add(dst, src, bsrc)
                    # q/k: out[f, t] ; lhsT = W tile slice, rhs = xT tile
                    for p in range(NPAIR):
                        for ch in range(2 * half, 2 * (half + 1)):
                            co = ch * 512 - half * (T // 2)
                            for (wsb, bsb, dstt) in ((wq_sb, bq_sb, qT),
                                                     (wk_sb, bk_sb, kTt)):
                                ps = pp.tile([128, 512], f32, tag="ps", name="psqk")
                                for d in range(ND):
                                    nc.tensor.matmul(
                                        ps[:],
                                        wsb[d][:, 128 * p:128 * (p + 1)],
                                        xs[d][:, co:co + 512],
                                        start=(d == 0), stop=(d == ND - 1))
                                nc.vector.tensor_scalar_add(
                                    dstt[p][:, 512 * ch:512 * (ch + 1)],
                                    ps[:], bsb[:, p:p + 1])

            with tc.tile_pool(name="xta", bufs=1) as xpool:
                qkv_half(xpool, 0)
            with tc.tile_pool(name="xtb", bufs=1) as xpool:
                qkv_half(xpool, 1)
            wts.__exit__(None, None, None)

            # ================= phase C: attention =================
            oTp = es.enter_context(tc.tile_pool(name="oTp", bufs=1))
            outT = [oTp.tile([128, T], f32r, tag=f"outT{p}", name=f"outT{p}")
                    for p in range(NPAIR)]
            with tc.tile_pool(name="ptp", bufs=4) as ptp, \
                 tc.tile_pool(name="nrm", bufs=4) as nrm, \
                 tc.tile_pool(name="psS", bufs=4, space="PSUM") as psS, \
                 tc.tile_pool(name="psO", bufs=2, space="PSUM") as psO:
                for p in range(NPAIR):
                    for c in range(NCH):
                        qs = 512 * c
                        nk = 4 * c + 4          # k-tiles for this chunk
                        po = [psO.tile([65, 512], f32, tag=f"psO{h}", name=f"po{h}")
                              for h in range(2)]
                        for i in range(nk):
                            dg = i - 4 * c      # >=0 on diagonal chunk
                            qo = max(dg, 0) * 128
                            w = 512 - qo
                            for h in range(2):
                                r0, r1 = 64 * h, 64 * (h + 1)
                                ss = psS.tile([128, 512], f32, tag="psS", name="ss")
                                nc.tensor.matmul(
                                    ss[:, :w],
                                    kTt[p][r0:r1, 128 * i:128 * (i + 1)],
                                    qT[p][r0:r1, qs + qo:qs + 512],
                                    start=True, stop=True)
                                pt = ptp.tile([128, 512], f32r, tag="pt", name="pt")
                                nc.scalar.activation(pt[:, qo:], ss[:, :w],
                                                     Exp, scale=0.125)
                                if dg >= 0:
                                    # zero strict upper triangle: keep f >= p
                                    nc.gpsimd.affine_select(
                                        out=pt[:, qo:qo + 128],
                                        in_=pt[:, qo:qo + 128],
                                        compare_op=mybir.AluOpType.is_ge,
                                        fill=0.0, base=0,
                                        pattern=[[1, 128]],
                                        channel_multiplier=-1)
                                hh = 2 * p + h
                                # dead q-columns [0:qo) are never written:
                                # the i==0 matmul always covers the full 512
                                # (qo==0 there), later partial-width matmuls
                                # accumulate only their live columns.
                                nc.tensor.matmul(
                                    po[h][:, qo:],
                                    vaug[:, VSTR * hh + 65 * i:
                                         VSTR * hh + 65 * i + 65],
                                    pt[:, qo:],
                                    start=(i == 0), stop=(i == nk - 1))
                        for h in range(2):
                            r0, r1 = 64 * h, 64 * (h + 1)
                            rec = nrm.tile([1, 512], f32, tag="rec", name="rec")
                            nc.vector.reciprocal(rec[:], po[h][64:65, :])
                            bc = nrm.tile([64, 512], f32, tag="bc", name="bc")
                            nc.gpsimd.partition_broadcast(bc[:], rec[:])
                            nc.vector.tensor_mul(
                                outT[p][r0:r1, qs:qs + 512],
                                po[h][0:64, :], bc[:])

            # ================= phase D: output projection =================
            with tc.tile_pool(name="wo", bufs=1) as wop, \
                 tc.tile_pool(name="ob", bufs=4) as obp, \
                 tc.tile_pool(name="psP", bufs=3, space="PSUM") as psP:
                wo_sb = [wop.tile([128, D], f32r, tag=f"wo{p}", name=f"wo{p}")
                         for p in range(NPAIR)]
                for p in range(NPAIR):
                    nc.gpsimd.dma_start(wo_sb[p][:],
                                        Wo[128 * p:128 * (p + 1), :])
                for tj in range(KT):
                    for n in range(2):
                        ps = psP.tile([128, 512], f32, tag="psP", name="pspr")
                        for p in range(NPAIR):
                            nc.tensor.matmul(
                                ps[:],
                                outT[p][:, 128 * tj:128 * (tj + 1)],
                                wo_sb[p][:, 512 * n:512 * (n + 1)],
                                start=(p == 0), stop=(p == NPAIR - 1))
                        ob = obp.tile([128, 512], f32, tag="ob", name="ob")
                        nc.any.tensor_copy(ob[:], ps[:])
                        nc.sync.dma_start(
                            y[128 * tj:128 * (tj + 1),
                              512 * n:512 * (n + 1)], ob[:])
    nc.compile()
    return nc


class _Runner:
    def __init__(self, nc):
        import jax
        from jax.sharding import Mesh, PartitionSpec, NamedSharding
        from jax.experimental.shard_map import shard_map
        import concourse.mybir as mybir
        from concourse.bass2jax import (_bass_exec_p, partition_id_tensor,
                                        install_neuronx_cc_hook)
        install_neuronx_cc_hook()
        self.jax = jax
        part = nc.partition_id_tensor.name if nc.partition_id_tensor else None
        in_names, out_names, out_avals = [], [], []
        for alloc in nc.m.functions[0].allocations:
            if not isinstance(alloc, mybir.MemoryLocationSet):
                continue
            name = alloc.memorylocations[0].name
            if alloc.kind == "ExternalInput":
                if name != part:
                    in_names.append(name)
            elif alloc.kind == "ExternalOutput":
                out_names.append(name)
                out_avals.append(jax.core.ShapedArray(
                    tuple(alloc.tensor_shape), mybir.dt.np(alloc.dtype)))
        self.in_names, self.out_names, self.out_avals = in_names, out_names, out_avals
        all_in = list(in_names) + list(out_names) + ([part] if part else [])

        def _body(*args):
            ops = list(args)
            if part:
                ops.append(partition_id_tensor())
            return tuple(_bass_exec_p.bind(
                *ops, out_avals=tuple(out_avals), in_names=tuple(all_in),
                out_names=tuple(out_names), lowering_input_output_aliases=(),
                sim_require_finite=True, sim_require_nnan=True, nc=nc))

        devices = jax.devices()[:NCORES]
        mesh = Mesh(np.asarray(devices), ("core",))
        nin = len(in_names) + len(out_names)
        self.fn = jax.jit(
            shard_map(_body, mesh=mesh,
                      in_specs=(PartitionSpec("core"),) * nin,
                      out_specs=(PartitionSpec("core"),) * len(out_names),
                      check_rep=False),
            keep_unused=True)
        self.sharding = NamedSharding(mesh, PartitionSpec("core"))

    def put_inputs(self, in_maps):
        args = []
        for name in self.in_names:
            cat = np.concatenate([np.asarray(m[name]) for m in in_maps], axis=0)
            args.append(self.jax.device_put(cat, self.sharding))
        for av in self.out_avals:
            z = np.zeros((NCORES * av.shape[0], *av.shape[1:]), av.dtype)
            args.append(self.jax.device_put(z, self.sharding))
        return args

    def run_np(self, args):
        outs = self.fn(*args)
        return [
            {n: np.asarray(outs[i]).reshape(NCORES, *self.out_avals[i].shape)[c]
             for i, n in enumerate(self.out_names)}
            for c in range(NCORES)
        ]


def _get_runner():
    if "r" not in _CACHE:
        nc = _build()
        _CACHE["nc"] = nc
        _CACHE["r"] = _Runner(nc)
    return _CACHE["r"]


def make_in_maps(x, Wqkv, bqkv, Wo, bo=None, mask=None):
    in_maps = []
    for c in range(NCORES):
        b, g = c // 2, c % 2
        sl = slice(g * FPC, (g + 1) * FPC)
        in_maps.append({
            "xT": np.ascontiguousarray(x[b].T),
            "Wq": np.ascontiguousarray(Wqkv[:, 0 * D:1 * D][:, sl]),
            "Wk": np.ascontiguousarray(Wqkv[:, 1 * D:2 * D][:, sl]),
            "Wv": np.ascontiguousarray(Wqkv[:, 2 * D:3 * D][:, sl]),
            "bq": np.ascontiguousarray(bqkv[0 * D:1 * D][sl]),
            "bk": np.ascontiguousarray(bqkv[1 * D:2 * D][sl]),
            "bv": np.ascontiguousarray(bqkv[2 * D:3 * D][sl]),
            "Wo": np.ascontiguousarray(Wo[sl, :]),
        })
    return in_maps


def kernel(x, Wqkv, bqkv, Wo, bo, mask=None, **_unused):
    x = np.asarray(x, dtype=np.float32)
    Wqkv = np.asarray(Wqkv, dtype=np.float32)
    bqkv = np.asarray(bqkv, dtype=np.float32)
    Wo = np.asarray(Wo, dtype=np.float32)
    bo = np.asarray(bo, dtype=np.float32)
    r = _get_runner()
    args = r.put_inputs(make_in_maps(x, Wqkv, bqkv, Wo))
    res = r.run_np(args)
    out = np.empty((B, T, D), dtype=np.float32)
    for b in range(B):
        out[b] = res[2 * b]["y"] + res[2 * b + 1]["y"] + bo
    return out


# revision 11
# speedup vs baseline: 1.0677x; 1.0677x over previous
"""Causal self-attention Trainium2 kernel.

Sharding: 8 cores = 4 batches x 2 head-groups (8 heads each).
Per-core dataflow (everything "transposed" so the contraction dim sits on
SBUF partitions):
  qT/kT  [64*2, T]  = Wq/Wk-slice.T-as-lhsT  @ xT          (fp32r matmuls)
  V      [T, 64*8]  = xT-as-lhsT @ Wv-slice, stored per (head, ktile) as
                      [128, 65] with a ones column (row-sum trick)
  ST     [k, q]     = kT-as-lhsT @ qT   (two heads row-packed in PE array)
  PT     = exp(ST/8) with causal lower-left structure (upper blocks skipped,
           diagonal blocks masked on GPSIMD)
  OT[65, q] += Vaug-as-lhsT @ PT        (row 64 accumulates softmax sums)
  outT   = OT[0:64] * bcast(1/OT[64])
  y      = outT-as-lhsT @ Wo-row-slice  (partial; host sums the 2 head-groups)
"""
import numpy as np

B, T, D, H = 4, 2048, 1024, 16
HD = D // H            # 64
NCORES = 8
HPC = 8                # heads per core
FPC = HPC * HD         # 512 feature cols per core
NPAIR = HPC // 2       # 4 head pairs
ND = D // 128          # 8 contraction tiles
KT = T // 128          # 16 k-tiles
NCH = T // 512         # 4 q-chunks

_CACHE = {}


def _build(phases="BCD"):
    import concourse.mybir as mybir
    import concourse.tile as tile
    from concourse import bacc

    f32 = mybir.dt.float32
    f32r = mybir.dt.float32r
    Exp = mybir.ActivationFunctionType.Exp
    mult = mybir.AluOpType.mult

    nc = bacc.Bacc("TRN2", target_bir_lowering=False, debug=False,
                   num_devices=NCORES)
    xT = nc.dram_tensor("xT", [D, T], f32, kind="ExternalInput")
    Wq = nc.dram_tensor("Wq", [D, FPC], f32, kind="ExternalInput")
    Wk = nc.dram_tensor("Wk", [D, FPC], f32, kind="ExternalInput")
    Wv = nc.dram_tensor("Wv", [D, FPC], f32, kind="ExternalInput")
    bq = nc.dram_tensor("bq", [FPC], f32, kind="ExternalInput")
    bk = nc.dram_tensor("bk", [FPC], f32, kind="ExternalInput")
    bv = nc.dram_tensor("bv", [FPC], f32, kind="ExternalInput")
    Wo = nc.dram_tensor("Wo", [FPC, D], f32, kind="ExternalInput")
    y = nc.dram_tensor("y", [T, D], f32, kind="ExternalOutput")

    VSTR = KT * 65     # per-head stride in Vaug free dim

    from contextlib import ExitStack
    with tile.TileContext(nc) as tc, ExitStack() as es:
        pers = es.enter_context(tc.tile_pool(name="pers", bufs=1))
        wts = tc.tile_pool(name="wts", bufs=1)
        wpool = wts.__enter__()
        if True:
            # ---- resident tiles ----
            wq_sb = [wpool.tile([128, FPC], f32r, tag=f"wq{d}", name=f"wq{d}") for d in range(ND)]
            wk_sb = [wpool.tile([128, FPC], f32r, tag=f"wk{d}", name=f"wk{d}") for d in range(ND)]
            wv_sb = [wpool.tile([128, FPC], f32r, tag=f"wv{d}", name=f"wv{d}") for d in range(ND)]
            qT = [pers.tile([128, T], f32r, tag=f"qT{p}", name=f"qT{p}") for p in range(NPAIR)]
            kTt = [pers.tile([128, T], f32r, tag=f"kT{p}", name=f"kT{p}") for p in range(NPAIR)]
            vaug = pers.tile([128, HPC * VSTR], f32r, tag="vaug")
            bq_sb = wpool.tile([128, NPAIR], f32, tag="bq")
            bk_sb = wpool.tile([128, NPAIR], f32, tag="bk")
            bv_row = wpool.tile([1, FPC], f32, tag="bvr")
            bv_bc = wpool.tile([128, FPC], f32, tag="bvb")

            for d in range(ND):
                nc.gpsimd.dma_start(wq_sb[d][:], Wq[128 * d:128 * (d + 1), :])
                nc.gpsimd.dma_start(wk_sb[d][:], Wk[128 * d:128 * (d + 1), :])
            for d in range(ND):
                nc.gpsimd.dma_start(wv_sb[d][:], Wv[128 * d:128 * (d + 1), :])
            nc.sync.dma_start(bq_sb[:], bq[:].rearrange("(t p) -> p t", p=128))
            nc.sync.dma_start(bk_sb[:], bk[:].rearrange("(t p) -> p t", p=128))
            nc.sync.dma_start(bv_row[:], bv[:])
            nc.gpsimd.partition_broadcast(bv_bc[:], bv_row[:])
            # ones columns of vaug (col 64 of each [128,65] block).
            # memset can't encode f32r, so memset f32 then cast-copy on DVE.
            ones_f32 = wpool.tile([128, KT], f32, tag="ones1", name="ones_f32")
            nc.vector.memset(ones_f32[:], 1.0)
            for h in range(HPC):
                ones_ap = vaug[:, VSTR * h:VSTR * (h + 1)].rearrange(
                    "p (k x) -> p k x", x=65)[:, :, 64:65]
                nc.vector.tensor_copy(ones_ap, ones_f32[:].rearrange(
                    "p (k x) -> p k x", x=1))

            # ================= phase B: qkv projections =================
            def qkv_half(xpool, half):
                xs = [xpool.tile([128, T // 2], f32r, tag=f"x{half}_{d}", name=f"x{half}_{d}")
                      for d in range(ND)]
                # chunked loads so the first q/k accumulation group can
                # start before the whole half arrives
                for cc in range(2):
                    for d in range(ND):
                        nc.gpsimd.dma_start(
                            xs[d][:, 512 * cc:512 * (cc + 1)],
                            xT[128 * d:128 * (d + 1),
                               half * (T // 2) + 512 * cc:
                               half * (T // 2) + 512 * (cc + 1)])
                with tc.tile_pool(name=f"psB{half}", bufs=4, space="PSUM") as pp:
                    # q/k first: attention scores only need qT/kT, so the
                    # ACT/PE attention pipeline can start before V is done
                    for p in range(NPAIR):
                        for ch in range(2 * half, 2 * (half + 1)):
                            co = ch * 512 - half * (T // 2)
                            for (wsb, bsb, dstt) in ((wq_sb, bq_sb, qT),
                                                     (wk_sb, bk_sb, kTt)):
                                ps = pp.tile([128, 512], f32, tag="ps", name="psqk")
                                for d in range(ND):
                                    nc.tensor.matmul(
                                        ps[:],
                                        wsb[d][:, 128 * p:128 * (p + 1)],
                                        xs[d][:, co:co + 512],
                                        start=(d == 0), stop=(d == ND - 1))
                                nc.vector.tensor_scalar_add(
                                    dstt[p][:, 512 * ch:512 * (ch + 1)],
                                    ps[:], bsb[:, p:p + 1])
                    # V: out[t, dv] ; lhsT = xT tile, rhs = Wv tile
                    for tj in range(8 * half, 8 * (half + 1)):
                        ps = pp.tile([128, FPC], f32, tag="ps", name="psv")
                        lo = tj * 128 - half * (T // 2)
                        for d in range(ND):
                            nc.tensor.matmul(ps[:], xs[d][:, lo:lo + 128],
                                             wv_sb[d][:],
                                             start=(d == 0), stop=(d == ND - 1))
                        # scatter per-head into vaug [p, 8, 64] + bias
                        dst = vaug[:].rearrange("p (h z) -> p h z", h=HPC)[
                            :, :, 65 * tj:65 * tj + HD]
                        src = ps[:].rearrange("p (h x) -> p h x", h=HPC)
                        bsrc = bv_bc[:].rearrange("p (h x) -> p h x", h=HPC)
                        nc.vector.tensor_add(dst, src, bsrc)

            with tc.tile_pool(name="xta", bufs=1) as xpool:
                qkv_half(xpool, 0)
            with tc.tile_pool(name="xtb", bufs=1) as xpool:
                qkv_half(xpool, 1)
            wts.__exit__(None, None, None)

            # ================= phase C: attention =================
            if "C" not in phases:
                return nc
            oTp = es.enter_context(tc.tile_pool(name="oTp", bufs=1))
            outT = [oTp.tile([128, T], f32r, tag=f"outT{p}", name=f"outT{p}")
                    for p in range(NPAIR)]
            with tc.tile_pool(name="wo", bufs=1) as wop, \
                 tc.tile_pool(name="ob", bufs=4) as obp, \
                 tc.tile_pool(name="ptp", bufs=4) as ptp, \
                 tc.tile_pool(name="nrm", bufs=4) as nrm, \
                 tc.tile_pool(name="psS", bufs=3, space="PSUM") as psS, \
                 tc.tile_pool(name="psO", bufs=1, space="PSUM") as psO:
                wo_sb = [wop.tile([128, D], f32r, tag=f"wo{p}", name=f"wo{p}")
                         for p in range(NPAIR)]
                for p in range(NPAIR):
                    nc.gpsimd.dma_start(wo_sb[p][:],
                                        Wo[128 * p:128 * (p + 1), :])
                for c in range(NCH):
                    qs = 512 * c
                    nk = 4 * c + 4          # k-tiles for this chunk
                    for p in range(NPAIR):
                        po = [psO.tile([65, 512], f32, tag=f"psO{h}",
                                       name=f"po{h}") for h in range(2)]
                        for i in range(nk):
                            dg = i - 4 * c  # >=0 on diagonal chunk
                            qo = max(dg, 0) * 128
                            w = 512 - qo
                            # both heads' scores into one 2-bank psum tile:
                            # head A cols [qo:512], head B cols [512+qo:1024]
                            ss = psS.tile([128, 1024], f32, tag="psS",
                                          name="ss")
                            pt = ptp.tile([128, 1024], f32r, tag="pt",
                                          name="pt")
                            for h in range(2):
                                r0, r1 = 64 * h, 64 * (h + 1)
                                nc.tensor.matmul(
                                    ss[:, 512 * h + qo:512 * (h + 1)],
                                    kTt[p][r0:r1, 128 * i:128 * (i + 1)],
                                    qT[p][r0:r1, qs + qo:qs + 512],
                                    start=True, stop=True)
                            sv = ss[:].rearrange("p (s x) -> p s x", s=2)[
                                :, :, qo:]
                            pv = pt[:].rearrange("p (s x) -> p s x", s=2)[
                                :, :, qo:]
                            nc.scalar.activation(pv, sv, Exp, scale=0.125)
                            if dg >= 0:
                                blk = pt[:].rearrange(
                                    "p (s x) -> p s x", s=2)[:, :, qo:qo + 128]
                                nc.gpsimd.affine_select(
                                    out=blk, in_=blk,
                                    compare_op=mybir.AluOpType.is_ge,
                                    fill=0.0, base=0,
                                    pattern=[[0, 2], [1, 128]],
                                    channel_multiplier=-1)
                            for h in range(2):
                                hh = 2 * p + h
                                nc.tensor.matmul(
                                    po[h][:, qo:],
                                    vaug[:, VSTR * hh + 65 * i:
                                         VSTR * hh + 65 * i + 65],
                                    pt[:, 512 * h + qo:512 * (h + 1)],
                                    start=(i == 0), stop=(i == nk - 1))
                        for h in range(2):
                            r0, r1 = 64 * h, 64 * (h + 1)
                            rec = nrm.tile([1, 512], f32, tag="rec",
                                           name="rec")
                            nc.vector.reciprocal(rec[:], po[h][64:65, :])
                            bc = nrm.tile([64, 512], f32, tag="bc", name="bc")
                            nc.gpsimd.partition_broadcast(bc[:], rec[:])
                            nc.vector.tensor_mul(
                                outT[p][r0:r1, qs:qs + 512],
                                po[h][0:64, :], bc[:])
                    # ---- phase D interleaved: project the finished chunk ----
                    if "D" not in phases:
                        continue
                    for tj in range(4 * c, 4 * (c + 1)):
                        for n in range(2):
                            ps = psS.tile([128, 512], f32, tag="psS",
                                          name="pspr")
                            for p in range(NPAIR):
                                nc.tensor.matmul(
                                    ps[:],
                                    outT[p][:, 128 * tj:128 * (tj + 1)],
                                    wo_sb[p][:, 512 * n:512 * (n + 1)],
                                    start=(p == 0), stop=(p == NPAIR - 1))
                            ob = obp.tile([128, 512], f32, tag="ob",
                                          name="ob")
                            nc.vector.tensor_copy(ob[:], ps[:])
                            nc.sync.dma_start(
                                y[128 * tj:128 * (tj + 1),
                                  512 * n:512 * (n + 1)], ob[:])
    nc.compile()
    return nc


class _Runner:
    def __init__(self, nc):
        import jax
        from jax.sharding import Mesh, PartitionSpec, NamedSharding
        from jax.experimental.shard_map import shard_map
        import concourse.mybir as mybir
        from concourse.bass2jax import (_bass_exec_p, partition_id_tensor,
                                        install_neuronx_cc_hook)
        install_neuronx_cc_hook()
        self.jax = jax
        part = nc.partition_id_tensor.name if nc.partition_id_tensor else None
        in_names, out_names, out_avals = [], [], []
        for alloc in nc.m.functions[0].allocations:
            if not isinstance(alloc, mybir.MemoryLocationSet):
                continue
            name = alloc.memorylocations[0].name
            if alloc.kind == "ExternalInput":
                if name != part:
                    in_names.append(name)
            elif alloc.kind == "ExternalOutput":
                out_names.append(name)
                out_avals.append(jax.core.ShapedArray(
                    tuple(alloc.tensor_shape), mybir.dt.np(alloc.dtype)))
        self.in_names, self.out_names, self.out_avals = in_names, out_names, out_avals
        all_in = list(in_names) + list(out_names) + ([part] if part else [])

        def _body(*args):
            ops = list(args)
            if part:
                ops.append(partition_id_tensor())
            return tuple(_bass_exec_p.bind(
                *ops, out_avals=tuple(out_avals), in_names=tuple(all_in),
                out_names=tuple(out_names), lowering_input_output_aliases=(),
                sim_require_finite=True, sim_require_nnan=True, nc=nc))

        devices = jax.devices()[:NCORES]
        mesh = Mesh(np.asarray(devices), ("core",))
        nin = len(in_names) + len(out_names)
        self.fn = jax.jit(
            shard_map(_body, mesh=mesh,
                      in_specs=(PartitionSpec("core"),) * nin,
                      out_specs=(PartitionSpec("core"),) * len(out_names),
                      check_rep=False),
            keep_unused=True)
        self.sharding = NamedSharding(mesh, PartitionSpec("core"))

    def put_inputs(self, in_maps):
        args = []
        for name in self.in_names:
            cat = np.concatenate([np.asarray(m[name]) for m in in_maps], axis=0)
            args.append(self.jax.device_put(cat, self.sharding))
        for av in self.out_avals:
            z = np.zeros((NCORES * av.shape[0], *av.shape[1:]), av.dtype)
            args.append(self.jax.device_put(z, self.sharding))
        return args

    def run_np(self, args):
        outs = self.fn(*args)
        return [
            {n: np.asarray(outs[i]).reshape(NCORES, *self.out_avals[i].shape)[c]
             for i, n in enumerate(self.out_names)}
            for c in range(NCORES)
        ]


def _get_runner():
    if "r" not in _CACHE:
        nc = _build()
        _CACHE["nc"] = nc
        _CACHE["r"] = _Runner(nc)
    return _CACHE["r"]


def make_in_maps(x, Wqkv, bqkv, Wo, bo=None, mask=None):
    in_maps = []
    for c in range(NCORES):
        b, g = c // 2, c % 2
        sl = slice(g * FPC, (g + 1) * FPC)
        in_maps.append({
            "xT": np.ascontiguousarray(x[b].T),
            "Wq": np.ascontiguousarray(Wqkv[:, 0 * D:1 * D][:, sl]),
            "Wk": np.ascontiguousarray(Wqkv[:, 1 * D:2 * D][:, sl]),
            "Wv": np.ascontiguousarray(Wqkv[:, 2 * D:3 * D][:, sl]),
            "bq": np.ascontiguousarray(bqkv[0 * D:1 * D][sl]),
            "bk": np.ascontiguousarray(bqkv[1 * D:2 * D][sl]),
            "bv": np.ascontiguousarray(bqkv[2 * D:3 * D][sl]),
            "Wo": np.ascontiguousarray(Wo[sl, :]),
        })
    return in_maps


def kernel(x, Wqkv, bqkv, Wo, bo, mask=None, **_unused):
    x = np.asarray(x, dtype=np.float32)
    Wqkv = np.asarray(Wqkv, dtype=np.float32)
    bqkv = np.asarray(bqkv, dtype=np.float32)
    Wo = np.asarray(Wo, dtype=np.float32)
    bo = np.asarray(bo, dtype=np.float32)
    r = _get_runner()
    args = r.put_inputs(make_in_maps(x, Wqkv, bqkv, Wo))
    res = r.run_np(args)
    out = np.empty((B, T, D), dtype=np.float32)
    for b in range(B):
        out[b] = res[2 * b]["y"] + res[2 * b + 1]["y"] + bo
    return out


# revision 15
# speedup vs baseline: 1.1834x; 1.1084x over previous
"""Causal self-attention Trainium2 kernel.

Sharding: 8 cores = 4 batches x 2 head-groups (8 heads each).
Per-core dataflow (everything "transposed" so the contraction dim sits on
SBUF partitions):
  qT/kT  [64*2, T]  = Wq/Wk-slice.T-as-lhsT  @ xT          (fp32r matmuls)
  V      [T, 64*8]  = xT-as-lhsT @ Wv-slice, stored per (head, ktile) as
                      [128, 65] with a ones column (row-sum trick)
  ST     [k, q]     = kT-as-lhsT @ qT   (two heads row-packed in PE array)
  PT     = exp(ST/8) with causal lower-left structure (upper blocks skipped,
           diagonal blocks masked on GPSIMD)
  OT[65, q] += Vaug-as-lhsT @ PT        (row 64 accumulates softmax sums)
  outT   = OT[0:64] * bcast(1/OT[64])
  y      = outT-as-lhsT @ Wo-row-slice  (partial; host sums the 2 head-groups)
"""
import numpy as np

B, T, D, H = 4, 2048, 1024, 16
HD = D // H            # 64
NCORES = 8
HPC = 8                # heads per core
FPC = HPC * HD         # 512 feature cols per core
NPAIR = HPC // 2       # 4 head pairs
ND = D // 128          # 8 contraction tiles
KT = T // 128          # 16 k-tiles
NCH = T // 512         # 4 q-chunks

_CACHE = {}


def _build(phases="BCD"):
    import concourse.mybir as mybir
    import concourse.tile as tile
    from concourse import bacc
    from contextlib import ExitStack

    f32 = mybir.dt.float32
    f32r = mybir.dt.float32r
    Exp = mybir.ActivationFunctionType.Exp

    nc = bacc.Bacc("TRN2", target_bir_lowering=False, debug=False,
                   num_devices=NCORES)
    xT = nc.dram_tensor("xT", [D, T], f32r, kind="ExternalInput")
    Wq = nc.dram_tensor("Wq", [D, FPC], f32r, kind="ExternalInput")
    Wk = nc.dram_tensor("Wk", [D, FPC], f32r, kind="ExternalInput")
    Wv = nc.dram_tensor("Wv", [D, FPC], f32r, kind="ExternalInput")
    bq = nc.dram_tensor("bq", [FPC], f32, kind="ExternalInput")
    bk = nc.dram_tensor("bk", [FPC], f32, kind="ExternalInput")
    bv = nc.dram_tensor("bv", [FPC], f32, kind="ExternalInput")
    Wo = nc.dram_tensor("Wo", [FPC, D], f32r, kind="ExternalInput")
    y = nc.dram_tensor("y", [T, D], f32, kind="ExternalOutput")

    VSTR = KT * 65     # per-head stride in vaug free dim

    with tile.TileContext(nc) as tc, ExitStack() as es:
        pers = es.enter_context(tc.tile_pool(name="pers", bufs=1))
        qkp = es.enter_context(tc.tile_pool(name="qkp", bufs=2))
        oTp = es.enter_context(tc.tile_pool(name="oTp", bufs=1))
        wqkp = es.enter_context(tc.tile_pool(name="wqkp", bufs=2))
        xsp = es.enter_context(tc.tile_pool(name="xsp", bufs=2))
        ptp = es.enter_context(tc.tile_pool(name="ptp", bufs=3))
        nrm = es.enter_context(tc.tile_pool(name="nrm", bufs=2))
        obp = es.enter_context(tc.tile_pool(name="obp", bufs=2))
        psA = es.enter_context(tc.tile_pool(name="psA", bufs=2, space="PSUM"))
        psS = es.enter_context(tc.tile_pool(name="psS", bufs=2, space="PSUM"))
        psO = es.enter_context(tc.tile_pool(name="psO", bufs=1, space="PSUM"))

        vaug = pers.tile([128, HPC * VSTR], f32r, tag="vaug")
        bq_sb = pers.tile([128, NPAIR], f32, tag="bq")
        bk_sb = pers.tile([128, NPAIR], f32, tag="bk")
        bv_row = pers.tile([1, FPC], f32, tag="bvr")
        bv_bc = pers.tile([128, FPC], f32, tag="bvb")
        outT = [oTp.tile([128, T], f32r, tag=f"outT{p}", name=f"outT{p}")
                for p in range(NPAIR)]

        nc.sync.dma_start(bq_sb[:], bq[:].rearrange("(t p) -> p t", p=128))
        nc.sync.dma_start(bk_sb[:], bk[:].rearrange("(t p) -> p t", p=128))
        nc.sync.dma_start(bv_row[:], bv[:])
        nc.gpsimd.partition_broadcast(bv_bc[:], bv_row[:])
        # ones columns of vaug (col 64 of each [128,65] block); memset can't
        # encode f32r so memset f32 then cast-copy on DVE
        ones_f32 = pers.tile([128, KT], f32, tag="ones1", name="ones_f32")
        nc.vector.memset(ones_f32[:], 1.0)
        for h in range(HPC):
            ones_ap = vaug[:, VSTR * h:VSTR * (h + 1)].rearrange(
                "p (k x) -> p k x", x=65)[:, :, 64:65]
            nc.vector.tensor_copy(ones_ap, ones_f32[:].rearrange(
                "p (k x) -> p k x", x=1))

        with tc.tile_pool(name="wvp", bufs=1) as wvp:
            wv_sb = [wvp.tile([128, FPC], f32r, tag=f"wv{d}", name=f"wv{d}")
                     for d in range(ND)]
            for d in range(ND):
                nc.sync.dma_start(wv_sb[d][:], Wv[128 * d:128 * (d + 1), :])

            for p in range(NPAIR):
                # ---- per-pair qkv projections, x streamed in quarters ----
                wq_p = wqkp.tile([128, D], f32r, tag="wq", name="wq_p")
                wk_p = wqkp.tile([128, D], f32r, tag="wk", name="wk_p")
                for d in range(ND):
                    nc.sync.dma_start(
                        wq_p[:, 128 * d:128 * (d + 1)],
                        Wq[128 * d:128 * (d + 1), 128 * p:128 * (p + 1)])
                    nc.sync.dma_start(
                        wk_p[:, 128 * d:128 * (d + 1)],
                        Wk[128 * d:128 * (d + 1), 128 * p:128 * (p + 1)])
                qTp = qkp.tile([128, T], f32r, tag="qT", name="qTp")
                kTp = qkp.tile([128, T], f32r, tag="kT", name="kTp")
                for ch in range(NCH):
                    xs = [xsp.tile([128, 512], f32r, tag=f"xs{d}",
                                   name=f"xs{d}") for d in range(ND)]
                    for d in range(ND):
                        nc.sync.dma_start(
                            xs[d][:],
                            xT[128 * d:128 * (d + 1),
                               512 * ch:512 * (ch + 1)])
                    for (wt, bsb, dst) in ((wq_p, bq_sb, qTp),
                                           (wk_p, bk_sb, kTp)):
                        ps = psA.tile([128, 512], f32, tag="psA", name="psqk")
                        for d in range(ND):
                            nc.tensor.matmul(
                                ps[:], wt[:, 128 * d:128 * (d + 1)],
                                xs[d][:], start=(d == 0), stop=(d == ND - 1))
                        nc.vector.tensor_scalar_add(
                            dst[:, 512 * ch:512 * (ch + 1)],
                            ps[:], bsb[:, p:p + 1])
                    if p == 0:
                        # V for all 8 heads, using this quarter's x
                        for tj in range(4 * ch, 4 * (ch + 1)):
                            ps = psA.tile([128, FPC], f32, tag="psA",
                                          name="psv")
                            lo = (tj - 4 * ch) * 128
                            for d in range(ND):
                                nc.tensor.matmul(
                                    ps[:], xs[d][:, lo:lo + 128], wv_sb[d][:],
                                    start=(d == 0), stop=(d == ND - 1))
                            dst = vaug[:].rearrange("p (h z) -> p h z",
                                                    h=HPC)[
                                :, :, 65 * tj:65 * tj + HD]
                            srcv = ps[:].rearrange("p (h x) -> p h x", h=HPC)
                            bsrc = bv_bc[:].rearrange("p (h x) -> p h x",
                                                      h=HPC)
                            nc.vector.tensor_add(dst, srcv, bsrc)

                # ---- attention for this pair ----
                if "C" not in phases:
                    continue
                for c in range(NCH):
                    qs = 512 * c
                    nk = 4 * c + 4
                    po = [psO.tile([65, 512], f32, tag=f"psO{h}",
                                   name=f"po{h}") for h in range(2)]
                    for i in range(nk):
                        dg = i - 4 * c
                        qo = max(dg, 0) * 128
                        # both heads' scores into one 2-bank psum tile
                        ss = psS.tile([128, 1024], f32, tag="psS", name="ss")
                        pt = ptp.tile([128, 1024], f32r, tag="pt", name="pt")
                        for h in range(2):
                            r0, r1 = 64 * h, 64 * (h + 1)
                            nc.tensor.matmul(
                                ss[:, 512 * h + qo:512 * (h + 1)],
                                kTp[r0:r1, 128 * i:128 * (i + 1)],
                                qTp[r0:r1, qs + qo:qs + 512],
                                start=True, stop=True)
                        sv = ss[:].rearrange("p (s x) -> p s x", s=2)[
                            :, :, qo:]
                        pv = pt[:].rearrange("p (s x) -> p s x", s=2)[
                            :, :, qo:]
                        nc.scalar.activation(pv, sv, Exp, scale=0.125)
                        if dg >= 0:
                            blk = pt[:].rearrange(
                                "p (s x) -> p s x", s=2)[:, :, qo:qo + 128]
                            nc.gpsimd.affine_select(
                                out=blk, in_=blk,
                                compare_op=mybir.AluOpType.is_ge,
                                fill=0.0, base=0,
                                pattern=[[0, 2], [1, 128]],
                                channel_multiplier=-1)
                        for h in range(2):
                            hh = 2 * p + h
                            nc.tensor.matmul(
                                po[h][:, qo:],
                                vaug[:, VSTR * hh + 65 * i:
                                     VSTR * hh + 65 * i + 65],
                                pt[:, 512 * h + qo:512 * (h + 1)],
                                start=(i == 0), stop=(i == nk - 1))
                    for h in range(2):
                        r0, r1 = 64 * h, 64 * (h + 1)
                        # single cheap copy frees the psO bank; the 3-op
                        # normalize chain then runs off-bank
                        og = nrm.tile([65, 512], f32, tag="og", name="og")
                        nc.vector.tensor_copy(og[:], po[h][:])
                        rec = nrm.tile([1, 512], f32, tag="rec", name="rec")
                        nc.vector.reciprocal(rec[:], og[64:65, :])
                        bc = nrm.tile([64, 512], f32, tag="bc", name="bc")
                        nc.gpsimd.partition_broadcast(bc[:], rec[:])
                        nc.vector.tensor_mul(
                            outT[p][r0:r1, qs:qs + 512],
                            og[0:64, :], bc[:])

        # ================= output projection =================
        if "D" not in phases:
            return nc
        with tc.tile_pool(name="wo", bufs=1) as wop:
            wo_sb = [wop.tile([128, D], f32r, tag=f"wo{p}", name=f"wo{p}")
                     for p in range(NPAIR)]
            for p in range(NPAIR):
                nc.sync.dma_start(wo_sb[p][:], Wo[128 * p:128 * (p + 1), :])
            for tj in range(KT):
                for n in range(2):
                    ps = psA.tile([128, 512], f32, tag="psA", name="pspr")
                    for p in range(NPAIR):
                        nc.tensor.matmul(
                            ps[:],
                            outT[p][:, 128 * tj:128 * (tj + 1)],
                            wo_sb[p][:, 512 * n:512 * (n + 1)],
                            start=(p == 0), stop=(p == NPAIR - 1))
                    ob = obp.tile([128, 512], f32, tag="ob", name="ob")
                    nc.vector.tensor_copy(ob[:], ps[:])
                    nc.sync.dma_start(
                        y[128 * tj:128 * (tj + 1),
                          512 * n:512 * (n + 1)], ob[:])
    nc.compile()
    return nc


class _Runner:
    def __init__(self, nc):
        import jax
        from jax.sharding import Mesh, PartitionSpec, NamedSharding
        from jax.experimental.shard_map import shard_map
        import concourse.mybir as mybir
        from concourse.bass2jax import (_bass_exec_p, partition_id_tensor,
                                        install_neuronx_cc_hook)
        install_neuronx_cc_hook()
        self.jax = jax
        part = nc.partition_id_tensor.name if nc.partition_id_tensor else None
        in_names, out_names, out_avals = [], [], []
        for alloc in nc.m.functions[0].allocations:
            if not isinstance(alloc, mybir.MemoryLocationSet):
                continue
            name = alloc.memorylocations[0].name
            if alloc.kind == "ExternalInput":
                if name != part:
                    in_names.append(name)
            elif alloc.kind == "ExternalOutput":
                out_names.append(name)
                out_avals.append(jax.core.ShapedArray(
                    tuple(alloc.tensor_shape), mybir.dt.np(alloc.dtype)))
        self.in_names, self.out_names, self.out_avals = in_names, out_names, out_avals
        all_in = list(in_names) + list(out_names) + ([part] if part else [])

        def _body(*args):
            ops = list(args)
            if part:
                ops.append(partition_id_tensor())
            return tuple(_bass_exec_p.bind(
                *ops, out_avals=tuple(out_avals), in_names=tuple(all_in),
                out_names=tuple(out_names), lowering_input_output_aliases=(),
                sim_require_finite=True, sim_require_nnan=True, nc=nc))

        devices = jax.devices()[:NCORES]
        mesh = Mesh(np.asarray(devices), ("core",))
        nin = len(in_names) + len(out_names)
        self.fn = jax.jit(
            shard_map(_body, mesh=mesh,
                      in_specs=(PartitionSpec("core"),) * nin,
                      out_specs=(PartitionSpec("core"),) * len(out_names),
                      check_rep=False),
            keep_unused=True)
        self.sharding = NamedSharding(mesh, PartitionSpec("core"))

    def put_inputs(self, in_maps):
        args = []
        for name in self.in_names:
            cat = np.concatenate([np.asarray(m[name]) for m in in_maps], axis=0)
            args.append(self.jax.device_put(cat, self.sharding))
        for av in self.out_avals:
            z = np.zeros((NCORES * av.shape[0], *av.shape[1:]), av.dtype)
            args.append(self.jax.device_put(z, self.sharding))
        return args

    def run_np(self, args):
        outs = self.fn(*args)
        return [
            {n: np.asarray(outs[i]).reshape(NCORES, *self.out_avals[i].shape)[c]
             for i, n in enumerate(self.out_names)}
            for c in range(NCORES)
        ]


def _get_runner():
    if "r" not in _CACHE:
        nc = _build()
        _CACHE["nc"] = nc
        _CACHE["r"] = _Runner(nc)
    return _CACHE["r"]


def _rne11(a):
    """Round fp32 to 11 mantissa bits, round-to-nearest-even (= hw fp32r)."""
    ai = np.ascontiguousarray(a, dtype=np.float32).view(np.uint32).astype(np.uint64)
    lsb = (ai >> 12) & 1
    out = (((ai + 2047 + lsb) >> 12) << 12).astype(np.uint32)
    return out.view(np.float32)


def make_in_maps(x, Wqkv, bqkv, Wo, bo=None, mask=None):
    in_maps = []
    for c in range(NCORES):
        b, g = c // 2, c % 2
        sl = slice(g * FPC, (g + 1) * FPC)
        in_maps.append({
            "xT": _rne11(x[b].T),
            "Wq": _rne11(Wqkv[:, 0 * D:1 * D][:, sl]),
            "Wk": _rne11(Wqkv[:, 1 * D:2 * D][:, sl]),
            "Wv": _rne11(Wqkv[:, 2 * D:3 * D][:, sl]),
            "bq": np.ascontiguousarray(bqkv[0 * D:1 * D][sl]),
            "bk": np.ascontiguousarray(bqkv[1 * D:2 * D][sl]),
            "bv": np.ascontiguousarray(bqkv[2 * D:3 * D][sl]),
            "Wo": _rne11(Wo[sl, :]),
        })
    return in_maps


def kernel(x, Wqkv, bqkv, Wo, bo, mask=None, **_unused):
    x = np.asarray(x, dtype=np.float32)
    Wqkv = np.asarray(Wqkv, dtype=np.float32)
    bqkv = np.asarray(bqkv, dtype=np.float32)
    Wo = np.asarray(Wo, dtype=np.float32)
    bo = np.asarray(bo, dtype=np.float32)
    r = _get_runner()
    args = r.put_inputs(make_in_maps(x, Wqkv, bqkv, Wo))
    res = r.run_np(args)
    out = np.empty((B, T, D), dtype=np.float32)
    for b in range(B):
        out[b] = res[2 * b]["y"] + res[2 * b + 1]["y"] + bo
    return out


# revision 18
# speedup vs baseline: 1.2309x; 1.0401x over previous
"""Causal self-attention Trainium2 kernel.

Sharding: 8 cores = 4 batches x 2 head-groups (8 heads each).
Per-core dataflow (everything "transposed" so the contraction dim sits on
SBUF partitions):
  qT/kT  [64*2, T]  = Wq/Wk-slice.T-as-lhsT  @ xT          (fp32r matmuls)
  V      [T, 64*8]  = xT-as-lhsT @ Wv-slice, stored per (head, ktile) as
                      [128, 65] with a ones column (row-sum trick)
  ST     [k, q]     = kT-as-lhsT @ qT   (two heads row-packed in PE array)
  PT     = exp(ST/8) with causal lower-left structure (upper blocks skipped,
           diagonal blocks masked on GPSIMD)
  OT[65, q] += Vaug-as-lhsT @ PT        (row 64 accumulates softmax sums)
  outT   = OT[0:64] * bcast(1/OT[64])
  y      = outT-as-lhsT @ Wo-row-slice  (partial; host sums the 2 head-groups)
"""
import numpy as np

B, T, D, H = 4, 2048, 1024, 16
HD = D // H            # 64
NCORES = 8
HPC = 8                # heads per core
FPC = HPC * HD         # 512 feature cols per core
NPAIR = HPC // 2       # 4 head pairs
ND = D // 128          # 8 contraction tiles
KT = T // 128          # 16 k-tiles
NCH = T // 512         # 4 q-chunks

_CACHE = {}


def _build(phases="BCD"):
    import concourse.mybir as mybir
    import concourse.tile as tile
    from concourse import bacc
    from contextlib import ExitStack

    f32 = mybir.dt.float32
    f32r = mybir.dt.float32r
    Exp = mybir.ActivationFunctionType.Exp

    nc = bacc.Bacc("TRN2", target_bir_lowering=False, debug=False,
                   num_devices=NCORES)
    # xT repacked host-side as [chunk, dtile, 128, 512]; Wq/Wk as
    # [pair, 128, 1024] so every load is a contiguous-row DMA
    xT = nc.dram_tensor("xT", [NCH, ND, 128, 512], f32r, kind="ExternalInput")
    Wq = nc.dram_tensor("Wq", [NPAIR, 128, D], f32r, kind="ExternalInput")
    Wk = nc.dram_tensor("Wk", [NPAIR, 128, D], f32r, kind="ExternalInput")
    Wv = nc.dram_tensor("Wv", [D, FPC], f32r, kind="ExternalInput")
    bq = nc.dram_tensor("bq", [FPC], f32, kind="ExternalInput")
    bk = nc.dram_tensor("bk", [FPC], f32, kind="ExternalInput")
    bv = nc.dram_tensor("bv", [FPC], f32, kind="ExternalInput")
    Wo = nc.dram_tensor("Wo", [FPC, D], f32r, kind="ExternalInput")
    y = nc.dram_tensor("y", [T, D], f32, kind="ExternalOutput")

    VSTR = KT * 65     # per-head stride in vaug free dim

    with tile.TileContext(nc) as tc, ExitStack() as es:
        pers = es.enter_context(tc.tile_pool(name="pers", bufs=1))
        qkp = es.enter_context(tc.tile_pool(name="qkp", bufs=2))
        oTp = es.enter_context(tc.tile_pool(name="oTp", bufs=1))
        wqkp = es.enter_context(tc.tile_pool(name="wqkp", bufs=2))
        xsp = es.enter_context(tc.tile_pool(name="xsp", bufs=2))
        ptp = es.enter_context(tc.tile_pool(name="ptp", bufs=3))
        nrm = es.enter_context(tc.tile_pool(name="nrm", bufs=2))
        obp = es.enter_context(tc.tile_pool(name="obp", bufs=3))
        psctx = ExitStack()
        psA = psctx.enter_context(tc.tile_pool(name="psA", bufs=2, space="PSUM"))
        psS = psctx.enter_context(tc.tile_pool(name="psS", bufs=2, space="PSUM"))
        psO = psctx.enter_context(tc.tile_pool(name="psO", bufs=1, space="PSUM"))

        vaug = pers.tile([128, HPC * VSTR], f32r, tag="vaug")
        bq_sb = pers.tile([128, NPAIR], f32, tag="bq")
        bk_sb = pers.tile([128, NPAIR], f32, tag="bk")
        bv_row = pers.tile([1, FPC], f32, tag="bvr")
        bv_bc = pers.tile([128, FPC], f32, tag="bvb")
        outT = [oTp.tile([128, T], f32r, tag=f"outT{p}", name=f"outT{p}")
                for p in range(NPAIR)]

        nc.sync.dma_start(bq_sb[:], bq[:].rearrange("(t p) -> p t", p=128))
        nc.sync.dma_start(bk_sb[:], bk[:].rearrange("(t p) -> p t", p=128))
        nc.sync.dma_start(bv_row[:], bv[:])
        nc.gpsimd.partition_broadcast(bv_bc[:], bv_row[:])
        # ones columns of vaug (col 64 of each [128,65] block); memset can't
        # encode f32r so memset f32 then cast-copy on DVE
        ones_f32 = pers.tile([128, KT], f32, tag="ones1", name="ones_f32")
        nc.vector.memset(ones_f32[:], 1.0)
        for h in range(HPC):
            ones_ap = vaug[:, VSTR * h:VSTR * (h + 1)].rearrange(
                "p (k x) -> p k x", x=65)[:, :, 64:65]
            nc.vector.tensor_copy(ones_ap, ones_f32[:].rearrange(
                "p (k x) -> p k x", x=1))

        with tc.tile_pool(name="wvp", bufs=1) as wvp:
            wv_sb = [wvp.tile([128, FPC], f32r, tag=f"wv{d}", name=f"wv{d}")
                     for d in range(ND)]
            wv_loaded = False

            for p in range(NPAIR):
                # ---- per-pair qkv projections, x streamed in quarters ----
                wq_p = wqkp.tile([128, D], f32r, tag="wq", name="wq_p")
                wk_p = wqkp.tile([128, D], f32r, tag="wk", name="wk_p")
                nc.sync.dma_start(wq_p[:], Wq[p])
                nc.sync.dma_start(wk_p[:], Wk[p])
                qTp = qkp.tile([128, T], f32r, tag="qT", name="qTp")
                kTp = qkp.tile([128, T], f32r, tag="kT", name="kTp")
                for ch in range(NCH):
                    xs = [xsp.tile([128, 512], f32r, tag=f"xs{d}",
                                   name=f"xs{d}") for d in range(ND)]
                    for d in range(ND):
                        nc.sync.dma_start(xs[d][:], xT[ch, d])
                    if not wv_loaded:
                        wv_loaded = True
                        for d in range(ND):
                            nc.sync.dma_start(wv_sb[d][:],
                                              Wv[128 * d:128 * (d + 1), :])
                    for (wt, bsb, dst) in ((wq_p, bq_sb, qTp),
                                           (wk_p, bk_sb, kTp)):
                        ps = psA.tile([128, 512], f32, tag="psA", name="psqk")
                        for d in range(ND):
                            nc.tensor.matmul(
                                ps[:], wt[:, 128 * d:128 * (d + 1)],
                                xs[d][:], start=(d == 0), stop=(d == ND - 1))
                        nc.vector.tensor_scalar_add(
                            dst[:, 512 * ch:512 * (ch + 1)],
                            ps[:], bsb[:, p:p + 1])
                    if p == 0:
                        # V for all 8 heads, using this quarter's x
                        for tj in range(4 * ch, 4 * (ch + 1)):
                            ps = psA.tile([128, FPC], f32, tag="psA",
                                          name="psv")
                            lo = (tj - 4 * ch) * 128
                            for d in range(ND):
                                nc.tensor.matmul(
                                    ps[:], xs[d][:, lo:lo + 128], wv_sb[d][:],
                                    start=(d == 0), stop=(d == ND - 1))
                            dst = vaug[:].rearrange("p (h z) -> p h z",
                                                    h=HPC)[
                                :, :, 65 * tj:65 * tj + HD]
                            srcv = ps[:].rearrange("p (h x) -> p h x", h=HPC)
                            bsrc = bv_bc[:].rearrange("p (h x) -> p h x",
                                                      h=HPC)
                            nc.vector.tensor_add(dst, srcv, bsrc)

                # ---- attention for this pair ----
                if "C" not in phases:
                    continue
                for c in range(NCH):
                    qs = 512 * c
                    nk = 4 * c + 4
                    po = [psO.tile([65, 512], f32, tag=f"psO{h}",
                                   name=f"po{h}") for h in range(2)]
                    for i in range(nk):
                        dg = i - 4 * c
                        qo = max(dg, 0) * 128
                        # both heads' scores into one 2-bank psum tile
                        ss = psS.tile([128, 1024], f32, tag="psS", name="ss")
                        pt = ptp.tile([128, 1024], f32r, tag="pt", name="pt")
                        for h in range(2):
                            r0, r1 = 64 * h, 64 * (h + 1)
                            nc.tensor.matmul(
                                ss[:, 512 * h + qo:512 * (h + 1)],
                                kTp[r0:r1, 128 * i:128 * (i + 1)],
                                qTp[r0:r1, qs + qo:qs + 512],
                                start=True, stop=True)
                        sv = ss[:].rearrange("p (s x) -> p s x", s=2)[
                            :, :, qo:]
                        pv = pt[:].rearrange("p (s x) -> p s x", s=2)[
                            :, :, qo:]
                        nc.scalar.activation(pv, sv, Exp, scale=0.125)
                        if dg >= 0:
                            blk = pt[:].rearrange(
                                "p (s x) -> p s x", s=2)[:, :, qo:qo + 128]
                            nc.gpsimd.affine_select(
                                out=blk, in_=blk,
                                compare_op=mybir.AluOpType.is_ge,
                                fill=0.0, base=0,
                                pattern=[[0, 2], [1, 128]],
                                channel_multiplier=-1)
                        for h in range(2):
                            hh = 2 * p + h
                            nc.tensor.matmul(
                                po[h][:, qo:],
                                vaug[:, VSTR * hh + 65 * i:
                                     VSTR * hh + 65 * i + 65],
                                pt[:, 512 * h + qo:512 * (h + 1)],
                                start=(i == 0), stop=(i == nk - 1))
                    for h in range(2):
                        r0, r1 = 64 * h, 64 * (h + 1)
                        # single cheap copy frees the psO bank; the 3-op
                        # normalize chain then runs off-bank
                        og = nrm.tile([65, 512], f32, tag="og", name="og")
                        nc.vector.tensor_copy(og[:], po[h][:])
                        rec = nrm.tile([1, 512], f32, tag="rec", name="rec")
                        nc.vector.reciprocal(rec[:], og[64:65, :])
                        bc = nrm.tile([64, 512], f32, tag="bc", name="bc")
                        nc.gpsimd.partition_broadcast(bc[:], rec[:])
                        nc.vector.tensor_mul(
                            outT[p][r0:r1, qs:qs + 512],
                            og[0:64, :], bc[:])

        psctx.close()
        # ================= output projection =================
        if "D" not in phases:
            return nc
        with tc.tile_pool(name="wo", bufs=1) as wop, \
             tc.tile_pool(name="psD", bufs=4, space="PSUM") as psD:
            wo_sb = [wop.tile([128, D], f32r, tag=f"wo{p}", name=f"wo{p}")
                     for p in range(NPAIR)]
            for p in range(NPAIR):
                nc.sync.dma_start(wo_sb[p][:], Wo[128 * p:128 * (p + 1), :])
            for tj in range(KT):
                for n in range(2):
                    ps = psD.tile([128, 512], f32, tag="psD", name="pspr")
                    for p in range(NPAIR):
                        nc.tensor.matmul(
                            ps[:],
                            outT[p][:, 128 * tj:128 * (tj + 1)],
                            wo_sb[p][:, 512 * n:512 * (n + 1)],
                            start=(p == 0), stop=(p == NPAIR - 1))
                    ob = obp.tile([128, 512], f32, tag="ob", name="ob")
                    nc.vector.tensor_copy(ob[:], ps[:])
                    nc.sync.dma_start(
                        y[128 * tj:128 * (tj + 1),
                          512 * n:512 * (n + 1)], ob[:])
    nc.compile()
    return nc


class _Runner:
    def __init__(self, nc):
        import jax
        from jax.sharding import Mesh, PartitionSpec, NamedSharding
        from jax.experimental.shard_map import shard_map
        import concourse.mybir as mybir
        from concourse.bass2jax import (_bass_exec_p, partition_id_tensor,
                                        install_neuronx_cc_hook)
        install_neuronx_cc_hook()
        self.jax = jax
        part = nc.partition_id_tensor.name if nc.partition_id_tensor else None
        in_names, out_names, out_avals = [], [], []
        for alloc in nc.m.functions[0].allocations:
            if not isinstance(alloc, mybir.MemoryLocationSet):
                continue
            name = alloc.memorylocations[0].name
            if alloc.kind == "ExternalInput":
                if name != part:
                    in_names.append(name)
            elif alloc.kind == "ExternalOutput":
                out_names.append(name)
                out_avals.append(jax.core.ShapedArray(
                    tuple(alloc.tensor_shape), mybir.dt.np(alloc.dtype)))
        self.in_names, self.out_names, self.out_avals = in_names, out_names, out_avals
        all_in = list(in_names) + list(out_names) + ([part] if part else [])

        def _body(*args):
            ops = list(args)
            if part:
                ops.append(partition_id_tensor())
            return tuple(_bass_exec_p.bind(
                *ops, out_avals=tuple(out_avals), in_names=tuple(all_in),
                out_names=tuple(out_names), lowering_input_output_aliases=(),
                sim_require_finite=True, sim_require_nnan=True, nc=nc))

        devices = jax.devices()[:NCORES]
        mesh = Mesh(np.asarray(devices), ("core",))
        nin = len(in_names) + len(out_names)
        self.fn = jax.jit(
            shard_map(_body, mesh=mesh,
                      in_specs=(PartitionSpec("core"),) * nin,
                      out_specs=(PartitionSpec("core"),) * len(out_names),
                      check_rep=False),
            keep_unused=True)
        self.sharding = NamedSharding(mesh, PartitionSpec("core"))

    def put_inputs(self, in_maps):
        args = []
        for name in self.in_names:
            cat = np.concatenate([np.asarray(m[name]) for m in in_maps], axis=0)
            args.append(self.jax.device_put(cat, self.sharding))
        for av in self.out_avals:
            z = np.zeros((NCORES * av.shape[0], *av.shape[1:]), av.dtype)
            args.append(self.jax.device_put(z, self.sharding))
        return args

    def run_np(self, args):
        outs = self.fn(*args)
        return [
            {n: np.asarray(outs[i]).reshape(NCORES, *self.out_avals[i].shape)[c]
             for i, n in enumerate(self.out_names)}
            for c in range(NCORES)
        ]


def _get_runner():
    if "r" not in _CACHE:
        nc = _build()
        _CACHE["nc"] = nc
        _CACHE["r"] = _Runner(nc)
    return _CACHE["r"]


def _rne11(a):
    """Round fp32 to 11 mantissa bits, round-to-nearest-even (= hw fp32r)."""
    ai = np.ascontiguousarray(a, dtype=np.float32).view(np.uint32).astype(np.uint64)
    lsb = (ai >> 12) & 1
    out = (((ai + 2047 + lsb) >> 12) << 12).astype(np.uint32)
    return out.view(np.float32)


def make_in_maps(x, Wqkv, bqkv, Wo, bo=None, mask=None):
    in_maps = []
    for c in range(NCORES):
        b, g = c // 2, c % 2
        sl = slice(g * FPC, (g + 1) * FPC)
        wqs = Wqkv[:, 0 * D:1 * D][:, sl].reshape(ND, 128, NPAIR, 128)
        wks = Wqkv[:, 1 * D:2 * D][:, sl].reshape(ND, 128, NPAIR, 128)
        in_maps.append({
            "xT": _rne11(x[b].reshape(NCH, 512, ND, 128).transpose(0, 2, 3, 1)),
            "Wq": _rne11(wqs.transpose(2, 1, 0, 3).reshape(NPAIR, 128, D)),
            "Wk": _rne11(wks.transpose(2, 1, 0, 3).reshape(NPAIR, 128, D)),
            "Wv": _rne11(Wqkv[:, 2 * D:3 * D][:, sl]),
            "bq": np.ascontiguousarray(bqkv[0 * D:1 * D][sl]),
            "bk": np.ascontiguousarray(bqkv[1 * D:2 * D][sl]),
            "bv": np.ascontiguousarray(bqkv[2 * D:3 * D][sl]),
            "Wo": _rne11(Wo[sl, :]),
        })
    return in_maps


def kernel(x, Wqkv, bqkv, Wo, bo, mask=None, **_unused):
    x = np.asarray(x, dtype=np.float32)
    Wqkv = np.asarray(Wqkv, dtype=np.float32)
    bqkv = np.asarray(bqkv, dtype=np.float32)
    Wo = np.asarray(Wo, dtype=np.float32)
    bo = np.asarray(bo, dtype=np.float32)
    r = _get_runner()
    args = r.put_inputs(make_in_maps(x, Wqkv, bqkv, Wo))
    res = r.run_np(args)
    out = np.empty((B, T, D), dtype=np.float32)
    for b in range(B):
        out[b] = res[2 * b]["y"] + res[2 * b + 1]["y"] + bo
    return out
